# revision 15
# baseline (speedup 1.0000x reference)
"""LocalAttention2d Bass kernel for 8 Trainium2 NeuronCores.

Strategy: pure data parallel over batch (8 batches/core).  The module attends
over an 8x8 window of data-dependent spatial positions per batch; the kernel
computes the window position on-device and gathers the 64 needed feature rows
per batch with a single indirect DMA from a host-pretransposed [B*H*W, D]
table.

Layout: gathered dest partition p <-> (b, khi) = (p//16, p%16), col block
t in 0..3 <-> window position k = khi*4 + t (k = i*8 + j with i = khi//2,
j = 4*(khi%2) + t).  The _prep_in_maps assert guarantees the window never
touches the padded NaN border, so for each p the four needed q rows are
CONSECUTIVE in the table and one [128,1]-offset indirect DMA with a
[128, 512] destination fetches everything (128 descriptors x 2KB,
HW-validated semantics: each offset fills its whole dest partition row from
a contiguous source span).

All per-(b,khi) quantities live directly in the 128-partition expanded
layout (one tiny PE matmul broadcasts exp(-z) to it), so the shift becomes a
handful of [128, 1..4] vector ops and the softmax/output selectors are host
constants.  sigmoid is computed via exp so the whole kernel uses a single
activation-table set.

Host-side work is limited to data-INdependent layout prep (transposes of q /
c_t / W_p, constant selector tables); every data-dependent step (p_t,
rounding, window indices, shift, softmax, output) runs on the NeuronCore.
"""

import sys

import numpy as np

try:
    import concourse.bass_utils as _bu
except ImportError:  # fresh grading dir: fall back to the repo checkout
    sys.path.insert(0, "/opt/trn_rl_repo")
    import concourse.bass_utils as _bu

import concourse.bacc as bacc
import concourse.bass as bass
import concourse.mybir as mybir
import concourse.tile as tile
import concourse.tile_sem_assignment as _tsa
from concourse.bass import IndirectOffsetOnAxis


def _patch_prep_tick():
    """Route gen_mode==1 SWDGE preps onto the Pool ENGINE proc (the same
    treatment tile gives user-synced remote-DMA preps).

    Tile's default books a prep on a DMASW lane whose semaphore is only
    bumped by the descriptor's completion sem -- but with a user `sem=`
    that slot carries the user's semaphore, so the epilogue drain waits on
    a DMASW sem that nothing ever fires (deadlocks both TimelineSim and
    hardware).  On the engine proc the prep's tick fires at desc-gen
    completion like any Pool engine op; DMA completion ordering is then
    the kernel's job via the explicit count=1 trigger protocol.
    """
    if getattr(_tsa.TileClockTick, "_prep_tick_patched", False):
        return
    orig = _tsa.TileClockTick._assign_tick

    def _assign_tick(self, inst):
        if (
            getattr(inst, "gen_mode", 0) == 1
            and isinstance(inst, _tsa.DMAInst)
            and inst.engine == mybir.EngineType.Pool
        ):
            eng_proc_idx = _tsa.ENGINE_TO_IDX[inst.engine]
            if inst.descendants or isinstance(inst, _tsa._DMA_OR_COLLECTIVE_TYPES):
                inst.bass_scheduled_tick = self.global_clock.advance(eng_proc_idx)
                inst.bass_scheduled_proc = eng_proc_idx
                inst.bass_scheduled_scope = self.scope_name
                self._proc_insts[self.root_scope_name][eng_proc_idx].append(inst)
            return
        return orig(self, inst)

    _tsa.TileClockTick._assign_tick = _assign_tick
    _tsa.TileClockTick._prep_tick_patched = True

B, D, H, W = 64, 128, 128, 128
CSZ = 256
R = 8                     # window rows == cols
NCORES = 8
BPC = B // NCORES         # batches per core
HW = H * W
NW = R * R                # 64 window positions
F32 = mybir.dt.float32
I32 = mybir.dt.int32

AOP = mybir.AluOpType
ACT = mybir.ActivationFunctionType

_USE_PREP_TRIGGER = True
_USE_S16 = True
_USE_ZERO = True

# aux_ct [128, 21]: 0:8 ct0 | 8:16 ct1 | 16:18 wp0 | 18:20 wp1 | 20:21 -ln128
ACT_W = 21
# aux_i  [128, 1] int32: gather-offset constant C0
# aux_sm [8, 133]: 0:128 sel | 128:132 ones4
SM_W = 132
# aux_bg [128, 301]: 0:128 wa0 | 128:256 wa1 | 256:288 pmask | 288:296 hsel
#   | 296:297 I1f | 297:301 J4f
BG_W = 301


def _build():
    _patch_prep_tick()
    nc = bacc.Bacc(
        "TRN2",
        target_bir_lowering=False,
        debug=False,
        num_devices=NCORES,
    )

    qhw = nc.dram_tensor("qhw", [BPC * HW, D], F32, kind="ExternalInput")
    aux_ct = nc.dram_tensor("aux_ct", [128, ACT_W], F32, kind="ExternalInput")
    aux_i = nc.dram_tensor("aux_i", [128, 1], I32, kind="ExternalInput")
    aux_sm = nc.dram_tensor("aux_sm", [BPC, SM_W], F32, kind="ExternalInput")
    aux_bg = nc.dram_tensor("aux_bg", [128, BG_W], F32, kind="ExternalInput")
    aux_s16 = nc.dram_tensor("aux_s16", [16, 16], mybir.dt.int16, kind="ExternalInput")
    out = nc.dram_tensor("out", [BPC, D], F32, kind="ExternalOutput")
    out_dma_sem = nc.alloc_semaphore(name="out_dma_sem")
    prep_sem = nc.alloc_semaphore(name="out_prep_sem")

    with tile.TileContext(nc) as tc:
        with (
            tc.tile_pool(name="sb", bufs=1) as sp,
            tc.tile_pool(name="ps", bufs=1, space="PSUM") as pp,
        ):
            # ---- input DMAs: tiny ct first (SP), consts on ACT queue ------
            a_ct = sp.tile([128, ACT_W], F32)
            nc.sync.dma_start(out=a_ct[:], in_=aux_ct[:])
            a_sm = sp.tile([BPC, SM_W], F32)
            nc.scalar.dma_start(out=a_sm[:], in_=aux_sm[:])
            a_i = sp.tile([128, 1], I32)
            nc.sync.dma_start(out=a_i[:], in_=aux_i[:])
            a_bg = sp.tile([128, BG_W], F32)
            nc.sync.dma_start(out=a_bg[:], in_=aux_bg[:])
            if _USE_S16:
                a_s16 = sp.tile([16, 16], mybir.dt.int16)
                nc.scalar.dma_start(out=a_s16[:], in_=aux_s16[:])
            if _USE_ZERO:
                # scatter-add output path: the destination must be zeroed
                # first (the zero DMA rides an idle HWDGE slot long before
                # the trigger)
                ztile = sp.tile([BPC, D], F32)
                nc.vector.memset(ztile[:], 0.0)
                zero_sem = nc.alloc_semaphore(name="out_zero_sem")
                nc.sync.dma_start(out=out[:], in_=ztile[:]).then_inc(zero_sem, 16)

            ct0 = a_ct[:, 0:8]
            ct1 = a_ct[:, 8:16]
            wp0 = a_ct[:, 16:18]
            wp1 = a_ct[:, 18:20]
            sel = a_sm[:, 0:128]
            ones4 = a_sm[:, 128:132]
            wa0 = a_bg[:, 0:128]
            wa1 = a_bg[:, 128:256]
            pmask = a_bg[:, 256:288]
            hsel = a_bg[:, 288:296]
            I1f = a_bg[:, 296:297]
            J4f = a_bg[:, 297:301]

            # ---- critical chain: p_t -> gather offsets -> gather ----------
            with tc.high_priority():
                # z = c_t @ W_p.T; eneg = exp(-z)/128 (bias = -ln 128)
                pt_p = pp.tile([BPC, 2], F32)
                nc.tensor.matmul(out=pt_p[:], lhsT=ct0, rhs=wp0, start=True, stop=False)
                nc.tensor.matmul(out=pt_p[:], lhsT=ct1, rhs=wp1, start=False, stop=True)
                eneg = sp.tile([BPC, 2], F32)
                nc.scalar.activation(
                    out=eneg[:], in_=pt_p[:], func=ACT.Exp, scale=-1.0,
                    bias=a_ct[0:BPC, 20:21],
                )
                # broadcast to the (b, khi) partition layout, then finish
                # p_t = 1/(1/128 + eneg) there
                enegB_p = pp.tile([128, 2], F32)
                nc.tensor.matmul(out=enegB_p[:], lhsT=sel, rhs=eneg[:], start=True, stop=True)
                u = sp.tile([128, 2], F32)
                nc.vector.tensor_scalar_add(u[:], enegB_p[:], 1.0 / H)
                ptB = sp.tile([128, 2], F32)
                nc.vector.reciprocal(ptB[:], u[:])
                priB = sp.tile([128, 2], I32)
                nc.vector.tensor_copy(priB[:], ptB[:])

                # gather offset: row0(p) = p_round_r*128 + p_round_c + C0
                # (C0 folds window offsets, the -1 pad shift and the batch
                #  base; border-free so all four rows are consecutive)
                t1 = sp.tile([128, 1], I32)
                nc.vector.tensor_scalar(
                    out=t1[:], in0=priB[:, 0:1], scalar1=7, scalar2=None,
                    op0=AOP.arith_shift_left,
                )
                t2 = sp.tile([128, 1], I32)
                nc.vector.tensor_tensor(out=t2[:], in0=t1[:], in1=priB[:, 1:2], op=AOP.add)
                offs = sp.tile([128, 1], I32)
                nc.vector.tensor_tensor(out=offs[:], in0=t2[:], in1=a_i[:], op=AOP.add)

                # 3+1 split: blocks t=0..2 first, then t=3 (offsets +3 rows),
                # so the per-block score pipeline starts ~400ns earlier and
                # drains with one block of work after the last gather lands
                qgA = sp.tile([128, 3 * D], F32)
                nc.gpsimd.indirect_dma_start(
                    out=qgA[:], out_offset=None, in_=qhw[:],
                    in_offset=IndirectOffsetOnAxis(ap=offs[:], axis=0),
                )
                offs3 = sp.tile([128, 1], I32)
                nc.vector.tensor_scalar(
                    out=offs3[:], in0=offs[:], scalar1=3, scalar2=None,
                    op0=AOP.add,
                )
                qgB = sp.tile([128, D], F32)
                nc.gpsimd.indirect_dma_start(
                    out=qgB[:], out_offset=None, in_=qhw[:],
                    in_offset=IndirectOffsetOnAxis(ap=offs3[:], axis=0),
                )

            def qg_t(t):  # gathered feature block for col t
                return qgA[:, t * D:(t + 1) * D] if t < 3 else qgB[:, 0:D]

            # ---- v[b, d] = sum_c c_t[b, c] W_a[c, d];  vB[p] = v[b(p)] ----
            # (tile_wait_until is a scheduling-pass hint: it keeps the 427ns
            #  v-matmuls from being ordered onto PE ahead of the critical
            #  expand matmul; no runtime timer is emitted)
            v_p = pp.tile([BPC, D], F32)
            with tc.tile_wait_until(0.005):
                nc.tensor.matmul(out=v_p[:], lhsT=ct0, rhs=wa0, start=True, stop=False)
                nc.tensor.matmul(out=v_p[:], lhsT=ct1, rhs=wa1, start=False, stop=True)
            v_s = sp.tile([BPC, D], F32)
            nc.scalar.activation(out=v_s[:], in_=v_p[:], func=ACT.Copy)
            vB_p = pp.tile([128, D], F32)
            nc.tensor.matmul(out=vB_p[:], lhsT=sel, rhs=v_s[:], start=True, stop=True)
            vB_s = sp.tile([128, D], F32)
            nc.scalar.activation(out=vB_s[:], in_=vB_p[:], func=ACT.Copy)
            # PE keep-warm: idle stretches reset the tensor engine to a slow
            # p-state; these scratch matmuls (results unused) keep the ramp
            # alive through the gather wait so the output matmuls run at
            # full clock.
            warm_p = pp.tile([128, D], F32, tag="warm")
            nc.tensor.matmul(out=warm_p[:], lhsT=sel, rhs=v_s[:], start=True, stop=True)
            warm2_p = pp.tile([BPC, D], F32, tag="warm2")
            nc.tensor.matmul(
                out=warm2_p[:], lhsT=qgA[:, 0:BPC], rhs=qgA[:, 0:D],
                start=True, stop=True,
            )

            # ---- negated shift, built under the gather --------------------
            # Dr = (p_round_r + I1) - p_t_r;  Dc[t] = (p_round_c + J4[t]) - p_t_c
            # negshf = -(Dr^2 + Dc^2)/8
            # (wait_until keeps these aux_bg-gated ops from being scheduled
            #  into the DVE queue ahead of the gather-offset chain)
            priBf = sp.tile([128, 2], F32)
            Dr = sp.tile([128, 1], F32)
            Dc = sp.tile([128, 4], F32)
            Dr2 = sp.tile([128, 1], F32)
            Dc2 = sp.tile([128, 4], F32)
            sm4 = sp.tile([128, 4], F32)
            negshf = sp.tile([128, 4], F32)
            with tc.tile_wait_until(0.006):
                nc.vector.tensor_copy(priBf[:], priB[:])
                nc.vector.scalar_tensor_tensor(
                    out=Dr[:], in0=priBf[:, 0:1], scalar=I1f, in1=ptB[:, 0:1],
                    op0=AOP.add, op1=AOP.subtract,
                )
                nc.vector.scalar_tensor_tensor(
                    out=Dc[:], in0=J4f, scalar=priBf[:, 1:2],
                    in1=ptB[:, 1:2].to_broadcast([128, 4]),
                    op0=AOP.add, op1=AOP.subtract,
                )
                nc.vector.tensor_tensor(out=Dr2[:], in0=Dr[:], in1=Dr[:], op=AOP.mult)
                nc.vector.tensor_tensor(out=Dc2[:], in0=Dc[:], in1=Dc[:], op=AOP.mult)
                nc.vector.tensor_tensor(
                    out=sm4[:], in0=Dc2[:], in1=Dr2[:].to_broadcast([128, 4]),
                    op=AOP.add,
                )
                nc.vector.tensor_scalar(
                    out=negshf[:], in0=sm4[:], scalar1=-0.125, scalar2=None,
                    op0=AOP.mult,
                )

            # ---- per-block pipeline: score -> exp -> rhs -> matmul --------
            s_all = sp.tile([128, 4], F32)
            e_t = sp.tile([128, 4], F32)
            rhs_all = sp.tile([128, 32], F32)
            outf_p = pp.tile([BPC, D], F32)
            s8_p = pp.tile([BPC, 1], F32)
            sinv = sp.tile([BPC, 1], F32)
            for t in range(4):
                # fused multiply + free-axis reduce on DVE (HW-validated:
                # scalar_tensor_tensor with accum_out; tensor_tensor_reduce
                # is NOT available in this runtime)
                pscr = sp.tile([128, D], F32, tag=f"pscr{t}")
                nc.vector.scalar_tensor_tensor(
                    out=pscr[:], in0=qg_t(t), scalar=1.0,
                    in1=vB_s[:], op0=AOP.mult, op1=AOP.mult,
                    accum_out=s_all[:, t:t + 1],
                )
                nc.scalar.activation(
                    out=e_t[:, t:t + 1], in_=s_all[:, t:t + 1], func=ACT.Exp,
                    bias=negshf[:, t:t + 1],
                )
                nc.vector.tensor_tensor(
                    out=rhs_all[:, t * BPC:(t + 1) * BPC],
                    in0=e_t[:, t:t + 1].to_broadcast([128, BPC]),
                    in1=pmask[:, t * BPC:(t + 1) * BPC],
                    op=AOP.mult,
                )
                # denominator accumulates per block in PSUM: 7ns PE ops that
                # never block the output matmuls, and sinv is ready before
                # the last matmul's semaphore
                nc.tensor.matmul(
                    out=s8_p[:], lhsT=hsel, rhs=e_t[:, t:t + 1],
                    start=(t == 0), stop=(t == 3),
                )
                if t == 3:
                    nc.vector.reciprocal(sinv[:], s8_p[:])
                nc.tensor.matmul(
                    out=outf_p[:],
                    lhsT=rhs_all[:, t * BPC:(t + 1) * BPC],
                    rhs=qg_t(t),
                    start=(t == 0), stop=(t == 3),
                )

            # normalized result lands in partitions 0..7 of a [128,1,D] tile;
            # partitions 8..127 scatter to index -1 (ignored)
            outf_s3 = sp.tile([128, 1, D], F32)
            nc.vector.tensor_scalar(
                out=outf_s3[0:BPC, 0, :], in0=outf_p[:], scalar1=sinv[:],
                scalar2=None, op0=AOP.mult,
            )
            # output write via SWDGE prepare+trigger: descriptors are
            # generated off the critical path (prep), the trigger only pays
            # the DMA-engine transfer + completion sem.  Explicit count=1
            # protocol: wait the prep sem (desc-gen committed) and the zero
            # DMA before triggering; the wait_until hint keeps these bare
            # semaphore waits from being front-loaded onto the Pool queue.
            if _USE_PREP_TRIGGER:
                nc.gpsimd.dma_scatter_add(
                    out_ap=out[:], in_ap=outf_s3[:], idxs_ap=a_s16[:, 0:1],
                    num_idxs=BPC, num_idxs_reg=BPC, elem_size=D,
                    prepare_only=True, sem=out_dma_sem,
                ).then_inc(prep_sem, 1)
                with tc.tile_wait_until(0.9):
                    nc.gpsimd.wait_ge(zero_sem, 16)
                    nc.gpsimd.wait_ge(prep_sem, 1)
                    nc.gpsimd.trigger_dma(count=1)
                    nc.gpsimd.wait_ge(out_dma_sem, 16)
            else:
                nc.sync.dma_start(out=out[:], in_=outf_s3[0:BPC, 0, :])

    nc.compile()
    return nc


_CACHE = {}


def _prep_in_maps(q, c_t, W_a, W_p):
    # Guard for the kernel's border-free fast path: every window index must
    # stay inside [1, 128] (pre-pad), i.e. p_round in [4, 124].  This holds
    # with large margin for the target input distribution; the check computes
    # nothing that feeds the output.
    _pt = 128.0 / (1.0 + np.exp(-(c_t.astype(np.float64) @ W_p.T.astype(np.float64))))
    _pr = np.rint(_pt)
    assert _pr.min() >= 4 and _pr.max() <= 124, (
        "window touches the NaN border; border-free kernel fast path invalid"
    )

    waT2 = W_a.astype(np.float32).reshape(2, 128, D)      # [2, 128, 128] row blocks
    wpT2 = W_p.T.astype(np.float32).reshape(2, 128, 2)    # [2, 128, 2] row blocks

    p = np.arange(128)
    bofp = p // 16                                        # batch of dest partition
    khi = p % 16
    iofp = khi // 2                                       # window row index i
    rofp = khi % 2                                        # j-pair selector

    # C0 = (i-4)*W + (4r-4) + b*HW: start row of the 4 consecutive q rows
    aux_i = ((iofp - 4) * W + 4 * rofp - 4 + bofp * HW).astype(np.int32)[:, None]

    bselm = (bofp[None, :] == np.arange(BPC)[:, None]).astype(np.float32)  # [8,128]
    aux_sm = np.zeros((BPC, SM_W), np.float32)
    aux_sm[:, 0:128] = bselm                              # sel
    aux_sm[:, 128:132] = 1.0                              # ones4

    aux_bg = np.zeros((128, BG_W), np.float32)
    aux_bg[:, 0:128] = waT2[0]
    aux_bg[:, 128:256] = waT2[1]
    pm = np.zeros((128, 4, BPC), np.float32)
    pm[p, :, bofp] = 1.0                                  # pmask[p, t, b] = (b==b(p))
    aux_bg[:, 256:288] = pm.reshape(128, 32)
    aux_bg[:, 288:296] = bselm.T                          # hsel[p, b]
    aux_bg[:, 296] = (iofp - 4).astype(np.float32)        # I1f
    aux_bg[:, 297:301] = (4 * rofp[:, None] + np.arange(4)[None, :] - 4).astype(
        np.float32
    )                                                     # J4f

    base_ct = np.zeros((128, ACT_W), np.float32)
    base_ct[:, 16:18] = wpT2[0]
    base_ct[:, 18:20] = wpT2[1]
    base_ct[:, 20] = -np.log(float(H))

    # scatter-add indices: batch row b -> out row b; -1 pads are ignored
    aux_s16 = np.full((16, 16), -1, np.int16)
    aux_s16[0:BPC, 0] = np.arange(BPC, dtype=np.int16)

    in_maps = []
    for c in range(NCORES):
        qs = q[c * BPC:(c + 1) * BPC]  # [BPC, D, H, W]
        qhw_np = np.ascontiguousarray(qs.transpose(0, 2, 3, 1)).reshape(BPC * HW, D)
        ctT_np = np.ascontiguousarray(c_t[c * BPC:(c + 1) * BPC].T)  # [CSZ, BPC]
        aux_ct = base_ct.copy()
        aux_ct[:, 0:8] = ctT_np[0:128]
        aux_ct[:, 8:16] = ctT_np[128:256]
        in_maps.append(
            {"qhw": qhw_np, "aux_ct": aux_ct, "aux_i": aux_i,
             "aux_sm": aux_sm, "aux_bg": aux_bg, "aux_s16": aux_s16}
        )
    return in_maps


def run(trace=False, **inputs):
    q = np.asarray(inputs["q"], dtype=np.float32)
    c_t = np.asarray(inputs["c_t"], dtype=np.float32)
    W_a = np.asarray(inputs["W_a"], dtype=np.float32)
    W_p = np.asarray(inputs["W_p"], dtype=np.float32)
    if "nc" not in _CACHE:
        _CACHE["nc"] = _build()
    in_maps = _prep_in_maps(q, c_t, W_a, W_p)
    res = _bu.run_bass_kernel_spmd(
        _CACHE["nc"], in_maps, core_ids=list(range(NCORES)), trace=trace
    )
    outp = np.concatenate([r["out"] for r in res.results], axis=0)
    return outp, res


def kernel(**inputs):
    outp, _ = run(trace=False, **inputs)
    return outp

